# revision 24
# baseline (speedup 1.0000x reference)
"""ChebConv2D (K1=K2=3) Trainium2 Bass kernel.

Data-parallel over batch (B=8) across 8 NeuronCores; per core the whole
per-batch computation runs on-chip.

Math (per batch, x: [N, N, C], N=200, C=32, OUT=64):
    out = U_0 + R_L(U_1) + R_{L^2}(U_2) + bias
    U_j = sum_i (A^i x) @ W'_ij^T      (Chebyshev folded into W' on host)

v3: fp16 data path (fp32 psum accumulation everywhere), DRAM-scratch
transpose with scratch laid out as the TT image (hop2 = 6 contiguous
reads), h-major S1 so S2 h=0 mixes overlap S1's second half, U half 0
held in one SBUF tensor for all n1.
"""

import numpy as np

import concourse.bass as bass
import concourse.mybir as mybir
from concourse import bacc
import concourse.tile as tile
from concourse import bass_utils


N = 200
C = 32
OUT = 64
B = 8
NC_HALF = 100
BLK = 8
NBLK = N // BLK
MCHUNKS = (N * C) // 128  # 50
F32 = mybir.dt.float32
F16 = mybir.dt.float16
MIXN = 192


def build_program():
    nc = bacc.Bacc("TRN2")

    x_d = nc.dram_tensor("x", [N, N * C], F16, kind="ExternalInput")
    g_d = nc.dram_tensor("g", [N, 3 * N], F16, kind="ExternalInput")
    ws_d = nc.dram_tensor("ws", [C * 3 + 1, MIXN], F16, kind="ExternalInput")
    lt1_d = nc.dram_tensor("lt1", [N, N], F16, kind="ExternalInput")
    lt2_d = nc.dram_tensor("lt2", [N, N], F16, kind="ExternalInput")
    ones_d = nc.dram_tensor("ones", [1, N * N], F16, kind="ExternalInput")
    out_d = nc.dram_tensor("out", [N, N, OUT], F32, kind="ExternalOutput")
    # transpose scratch, laid out as the TT image: [i][d][n2][n1]
    scr_d = nc.dram_tensor("scr", [C, N, 3, N], F16, kind="Internal")

    with tile.TileContext(nc) as tc:
        with (
            tc.tile_pool(name="const", bufs=1) as constp,
            tc.tile_pool(name="tt", bufs=1) as ttp,
            tc.tile_pool(name="u0", bufs=1) as u0p,
        ):
            g_t = []
            lt_t = {}
            for t in range(2):
                g = constp.tile([NC_HALF, 3 * N], F16, tag=f"g{t}")
                nc.sync.dma_start(g[:], g_d[t * NC_HALF:(t + 1) * NC_HALF, :])
                g_t.append(g)
                for j in (1, 2):
                    lt = constp.tile([NC_HALF, N], F16, tag=f"lt{j}{t}")
                    src = lt1_d if j == 1 else lt2_d
                    nc.sync.dma_start(lt[:], src[t * NC_HALF:(t + 1) * NC_HALF, :])
                    lt_t[(j, t)] = lt
            ws = constp.tile([C * 3 + 1, MIXN], F16, tag="ws")
            nc.sync.dma_start(ws[:], ws_d[:, :])

            TT = ttp.tile([C * 3 + 1, N * N], F16, tag="TT")
            nc.sync.dma_start(TT[96:97, :], ones_d[:, :])
            TT3 = TT[:].rearrange("p (a b) -> p a b", b=N)

            # U half 0 for all n1: [n2 0..99, n1*192 + (j,o)]
            UC0 = u0p.tile([NC_HALF, N * MIXN], F16, tag="UC0")

            XCH = 5
            with (
                tc.tile_pool(name="xa", bufs=4) as xap,
                tc.tile_pool(name="sg", bufs=8) as sgp,
                tc.tile_pool(name="uc", bufs=4) as ucp,
                tc.tile_pool(name="ob", bufs=4) as obp,
                tc.tile_pool(name="psU", bufs=2, space="PSUM") as psup,
            ):
                # ---- S1 (h-major) + hop2 per half ----
                psap_cm = tc.tile_pool(name="psA", bufs=3, space="PSUM")
                psap = psap_cm.__enter__()
                xt_big = [None, None]

                xq = {}

                def x_load(m):
                        for t in range(2):
                            xm = xap.tile([NC_HALF, XCH * 128], F16,
                                          tag=f"xm{t}", name=f"xm{t}_{m}")
                            nc.scalar.dma_start(
                                xm[:], x_d[t * NC_HALF:(t + 1) * NC_HALF,
                                           m * 128:(m + XCH) * 128])
                            xq[(t, m)] = xm

                def s1_chunk(m):
                        if m % XCH == 0:
                            if (0, m) not in xq:
                                x_load(m)
                            for t in range(2):
                                xt_big[t] = xq[(t, m)]
                        mm = m % XCH
                        psa = psap.tile([128, 300], F32, tag="psa")
                        psb = psap.tile([128, 300], F32, tag="psb")
                        for t in range(2):
                            lhsT = xt_big[t][:, mm * 128:(mm + 1) * 128]
                            nc.tensor.matmul(psa[:], lhsT, g_t[t][:, 0:300],
                                             start=(t == 0), stop=(t == 1))
                            nc.tensor.matmul(psb[:], lhsT, g_t[t][:, 300:600],
                                             start=(t == 0), stop=(t == 1))
                        sc = sgp.tile([128, 600], F16, tag="sc")
                        nc.vector.tensor_copy(sc[:, 0:300], psa[:])
                        nc.scalar.copy(sc[:, 300:600], psb[:])
                        # hop1: one DMA -> scratch [d][n2][i][n1]
                        dst = scr_d[:, 4 * m:4 * m + 4, :, :]
                        dst = dst.rearrange("d r i b -> r d (i b)")
                        nc.sync.dma_start(dst, sc[:, :])

                def hop2(h):
                    # 3 strided reads on the scalar HWDGE ring
                    for i in range(3):
                        src = scr_d[:, h * NC_HALF:(h + 1) * NC_HALF, i, :]
                        dst = TT3[i * 32:(i + 1) * 32,
                                  h * NC_HALF:(h + 1) * NC_HALF, :]
                        nc.scalar.dma_start(dst, src)

                def s2h0_pair(p2):
                    psu = psup.tile([NC_HALF, 2 * MIXN], F32, tag="psu",
                                    name=f"psu0_{p2}")
                    for q in range(2):
                        n1 = p2 * 2 + q
                        lhsT = TT3[0:97, 0:NC_HALF, n1:n1 + 1]
                        nc.tensor.matmul(psu[:, q * MIXN:(q + 1) * MIXN],
                                         lhsT, ws[:], start=True, stop=True)
                    dst = UC0[:].rearrange("p (n f) -> p n f", f=MIXN)[
                        :, p2 * 2:p2 * 2 + 2, :]
                    psu3 = psu[:].rearrange("p (q f) -> p q f", f=MIXN)
                    if p2 % 2 == 0:
                        nc.vector.tensor_copy(dst, psu3)
                    else:
                        nc.scalar.copy(dst, psu3)

                # S1 half 0, then interleave S1 half 1 with S2 h=0
                for m in range(25):
                    s1_chunk(m)
                x_load(25)
                x_load(30)
                hop2(0)
                for k in range(25):
                    s1_chunk(25 + k)
                    for p2 in range(2 * k, 2 * k + 2):
                        s2h0_pair(p2)
                hop2(1)
                for p2 in range(50, 100):
                    s2h0_pair(p2)
                psap_cm.__exit__(None, None, None)

                # ---- S2 h=1 + S3 per block ----
                UC03 = UC0[:].rearrange("p (n f) -> p n f", f=MIXN)
                psop = ctx_psop = tc.tile_pool(name="psO", bufs=2, space="PSUM")
                psop = psop.__enter__()
                for blk in range(NBLK):
                    uc1 = ucp.tile([NC_HALF, BLK * MIXN], F16, tag="uc1",
                                   name=f"uc1_{blk}")
                    for bi2 in range(BLK // 2):
                        psu = psup.tile([NC_HALF, 2 * MIXN], F32, tag="psu")
                        for q in range(2):
                            n1 = blk * BLK + bi2 * 2 + q
                            lhsT = TT3[0:97, NC_HALF:N, n1:n1 + 1]
                            nc.tensor.matmul(psu[:, q * MIXN:(q + 1) * MIXN],
                                             lhsT, ws[:], start=True, stop=True)
                        dst = uc1[:].rearrange("p (n f) -> p n f", f=MIXN)[
                            :, bi2 * 2:bi2 * 2 + 2, :]
                        psu3 = psu[:].rearrange("p (q f) -> p q f", f=MIXN)
                        if bi2 % 2 == 0:
                            nc.vector.tensor_copy(dst, psu3)
                        else:
                            nc.scalar.copy(dst, psu3)
                    uc13 = uc1[:].rearrange("p (n f) -> p n f", f=MIXN)
                    for m2 in range(2):
                        pso = psop.tile([NC_HALF, BLK * OUT], F32, tag="pso")
                        k = 0
                        for j in (1, 2):
                            for h in range(2):
                                lhsT = lt_t[(j, h)][:, m2 * NC_HALF:(m2 + 1) * NC_HALF]
                                u3 = UC03 if h == 0 else uc13
                                if h == 0:
                                    rhs = u3[:, blk * BLK:(blk + 1) * BLK,
                                             j * OUT:(j + 1) * OUT]
                                else:
                                    rhs = u3[:, :, j * OUT:(j + 1) * OUT]
                                nc.tensor.matmul(pso[:], lhsT, rhs,
                                                 start=(k == 0), stop=(k == 3))
                                k += 1
                        pso3 = pso[:].rearrange("p (n o) -> p n o", o=OUT)
                        u0s = (UC03 if m2 == 0 else uc13)
                        if m2 == 0:
                            u0 = u0s[:, blk * BLK:(blk + 1) * BLK, 0:OUT]
                        else:
                            u0 = u0s[:, :, 0:OUT]
                        ob = obp.tile([NC_HALF, BLK * OUT], F32, tag="ob")
                        ob3 = ob[:].rearrange("p (n o) -> p n o", o=OUT)
                        nc.vector.tensor_add(ob3, pso3, u0)
                        dst = out_d[blk * BLK:(blk + 1) * BLK,
                                    m2 * NC_HALF:(m2 + 1) * NC_HALF, :]
                        nc.scalar.dma_start(dst.rearrange("a b c -> b a c"), ob3)
                ctx_psop.__exit__(None, None, None)
    nc.compile()
    return nc


def _host_inputs(adj, weight, bias):
    adj = np.asarray(adj, np.float64)
    weight = np.asarray(weight, np.float64)
    bias = np.asarray(bias, np.float64)
    n = adj.shape[0]
    A = adj * (1.0 - np.eye(n))
    d0 = A.sum(0) ** -0.5
    d1 = A.sum(1) ** -0.5
    d0[np.isinf(d0)] = 0.0
    d1[np.isinf(d1)] = 0.0
    L = d0[:, None] * A * d1[None, :]
    L2 = L @ L

    p = np.array([[1.0, 0, 0], [0, 1.0, 0], [-1.0, 0, 2.0]])
    W = weight.reshape(OUT, 3, 3, C)
    Wp = np.einsum("ai,bj,oabc->ijoc", p, p, W)

    G = np.concatenate([np.eye(n), L, L2], axis=1)
    WS = np.zeros((3 * C + 1, MIXN))
    for i in range(3):
        for j in range(3):
            WS[i * C:(i + 1) * C, j * OUT:(j + 1) * OUT] = Wp[i, j].T
    WS[96, 0:OUT] = bias
    ones = np.ones((1, n * n))
    return (G.astype(np.float16), WS.astype(np.float16),
            np.ascontiguousarray(L.T).astype(np.float16),
            np.ascontiguousarray(L2.T).astype(np.float16),
            ones.astype(np.float16))


_PROGRAM = None


def kernel(x, adj, weight, bias):
    global _PROGRAM
    x = np.asarray(x)
    G, WS, LT1, LT2, ONES = _host_inputs(adj, weight, bias)
    if _PROGRAM is None:
        _PROGRAM = build_program()
    nc = _PROGRAM
    in_maps = []
    for b in range(B):
        in_maps.append({
            "x": np.ascontiguousarray(x[b].reshape(N, N * C)).astype(np.float16),
            "g": G, "ws": WS, "lt1": LT1, "lt2": LT2, "ones": ONES,
        })
    res = bass_utils.run_bass_kernel_spmd(nc, in_maps, core_ids=list(range(B)))
    out = np.stack([res.results[b]["out"] for b in range(B)], axis=0)
    return out.astype(np.float32)
